# revision 3
# baseline (speedup 1.0000x reference)
"""BitLinear fake-quant GEMM on 8 trn2 NeuronCores, data-parallel over batch.

Per core: y[s,o] = round(clip(x/a_scale*127)) @ clip(round(w/w_scale),-1,1)^T
          * (w_scale * a_scale / 127),  a_scale = rowmax|x| + eps.

Quantized activations are integers |v|<=127 and weights are ternary, so a
bf16 matmul with fp32 PSUM accumulation is exact integer arithmetic.
"""

import os
import sys

import numpy as np

sys.path.insert(0, "/opt/trn_rl_repo")

import concourse.bacc as bacc
import concourse.mybir as mybir
import concourse.tile as tile
from concourse.bass_utils import run_bass_kernel_spmd

F32 = mybir.dt.float32
BF16 = mybir.dt.bfloat16
AF = mybir.ActivationFunctionType
ALU = mybir.AluOpType

B = 8      # batches == cores
S = 4096   # rows per core
D = 1024   # in features (contraction)
O = 1024   # out features
P = 128
GA = 4     # s-tiles per DMA group
KB = D // P
RND = 12582912.0  # 1.5*2**23: (z+RND)-RND == round-half-even(z) for |z|<2**22
EPS = 1e-8

_CACHE = {}
TRACE_DIR = None


def _build(s_rows=S):
    nt = s_rows // P
    ng = nt // GA
    nc = bacc.Bacc("TRN2", target_bir_lowering=False, debug=False)
    x_d = nc.dram_tensor("x", [s_rows, D], F32, kind="ExternalInput")
    w_d = nc.dram_tensor("weight", [O, D], F32, kind="ExternalInput")
    wsc_d = nc.dram_tensor("wsc", [1, 2], F32, kind="ExternalInput")
    y_d = nc.dram_tensor("y", [s_rows, O], F32, kind="ExternalOutput")
    xa, wa, sca, ya = x_d.ap(), w_d.ap(), wsc_d.ap(), y_d.ap()

    with tile.TileContext(nc) as tc:
        with (
            tc.tile_pool(name="wraw", bufs=1) as wraw_p,
            tc.tile_pool(name="wq", bufs=2) as wq_p,
            tc.tile_pool(name="wqT", bufs=1) as wqT_p,
            tc.tile_pool(name="xg", bufs=3) as xg_p,
            tc.tile_pool(name="stat", bufs=4) as stat_p,
            tc.tile_pool(name="quant", bufs=3) as q_p,
            tc.tile_pool(name="aqT", bufs=3) as aqT_p,
            tc.tile_pool(name="yout", bufs=3) as y_p,
            tc.tile_pool(name="psum", bufs=2, space="PSUM") as ps_p,
        ):
            # scalar broadcast: wsc = [1/w_scale, w_scale/127] -> all partitions
            wsc0 = wraw_p.tile([P, 2], F32, tag="wsc0")
            nc.sync.dma_start(out=wsc0[0:1, :], in_=sca[:, :])
            wscb = wraw_p.tile([P, 2], F32, tag="wscb")
            nc.gpsimd.partition_broadcast(wscb[:, :], wsc0[0:1, :], channels=P)
            recw_b = wscb[:, 0:1]
            ws127_b = wscb[:, 1:2]

            # weight: quantize to ternary bf16, then transpose to [i, o]
            w_sb = wraw_p.tile([P, KB, D], F32, tag="wraw")  # [p, o-blk, i]
            nc.sync.dma_start(out=w_sb[:], in_=wa.rearrange("(a p) d -> p a d", p=P))
            wqT = wqT_p.tile([P, KB, O], BF16)  # [i-in-blk, i-blk, o]
            for k in range(KB):
                tw = wq_p.tile([P, D], F32, tag="tw")
                nc.scalar.activation(tw[:], w_sb[:, k, :], AF.Copy, bias=RND, scale=recw_b)
                tw2 = wq_p.tile([P, D], F32, tag="tw2")
                nc.vector.tensor_scalar(tw2[:], tw[:], RND, 1.0, ALU.subtract, ALU.min)
                wq = wq_p.tile([P, D], BF16, tag="wq")
                nc.vector.tensor_scalar(wq[:], tw2[:], -1.0, None, ALU.max)
                for b2 in range(KB):
                    nc.sync.dma_start_transpose(
                        wqT[:, b2, k * P:(k + 1) * P], wq[:, b2 * P:(b2 + 1) * P]
                    )

            for g in range(ng):
                xg = xg_p.tile([P, GA, D], F32)
                nc.sync.dma_start(
                    out=xg[:],
                    in_=xa[g * GA * P:(g + 1) * GA * P, :].rearrange("(a p) d -> p a d", p=P),
                )
                st = stat_p.tile([P, GA], F32, tag="st")
                nc.vector.tensor_reduce(
                    st[:], xg[:], mybir.AxisListType.X, ALU.max, apply_absolute_value=True
                )
                ga_t = stat_p.tile([P, GA], F32, tag="ga")
                nc.vector.tensor_scalar(ga_t[:], st[:], EPS, None, ALU.add)
                rec = stat_p.tile([P, GA], F32, tag="rec")
                nc.vector.reciprocal(rec[:], ga_t[:])
                rec127 = stat_p.tile([P, GA], F32, tag="rec127")
                nc.vector.tensor_scalar(rec127[:], rec[:], 127.0, None, ALU.mult)
                epi = stat_p.tile([P, GA], F32, tag="epi")
                nc.vector.tensor_scalar(epi[:], ga_t[:], ws127_b, None, ALU.mult)

                for a in range(GA):
                    t = g * GA + a
                    tq = q_p.tile([P, D], F32, tag="tq")
                    nc.scalar.activation(
                        tq[:], xg[:, a, :], AF.Copy, bias=RND, scale=rec127[:, a:a + 1]
                    )
                    aq = q_p.tile([P, D], BF16, tag="aq")
                    nc.vector.tensor_scalar(aq[:], tq[:], RND, None, ALU.subtract)
                    aqT = aqT_p.tile([P, KB, P], BF16)
                    for b2 in range(KB):
                        nc.sync.dma_start_transpose(aqT[:, b2, :], aq[:, b2 * P:(b2 + 1) * P])
                    yt = ps_p.tile([P, O], F32)
                    for b2 in range(KB):
                        nc.tensor.matmul(
                            yt[:, 0:512], aqT[:, b2, :], wqT[:, b2, 0:512],
                            start=(b2 == 0), stop=(b2 == KB - 1),
                        )
                        nc.tensor.matmul(
                            yt[:, 512:1024], aqT[:, b2, :], wqT[:, b2, 512:1024],
                            start=(b2 == 0), stop=(b2 == KB - 1),
                        )
                    ysb = y_p.tile([P, O], F32)
                    nc.scalar.activation(ysb[:], yt[:], AF.Copy, bias=0.0, scale=epi[:, a:a + 1])
                    nc.sync.dma_start(out=ya[t * P:(t + 1) * P, :], in_=ysb[:])
    nc.compile()
    return nc


def _scales(weight):
    # w_scale in fp64 then rounded, mirroring fp32 `mean(|w|) + eps` as closely
    # as any fp32 summation order allows.
    m = np.abs(weight.astype(np.float64)).mean()
    ws = np.float32(np.float32(m) + np.float32(EPS))
    recw = np.float32(1.0 / np.float64(ws))
    ws127 = np.float32(np.float64(ws) / 127.0)
    return np.array([[recw, ws127]], dtype=np.float32)


def kernel(x, weight):
    x = np.ascontiguousarray(np.asarray(x), dtype=np.float32)
    weight = np.ascontiguousarray(np.asarray(weight), dtype=np.float32)
    assert x.shape == (B, S, D) and weight.shape == (O, D)
    nc = _CACHE.get("nc")
    if nc is None:
        nc = _CACHE["nc"] = _build()
    wsc = _scales(weight)
    in_maps = [{"x": x[c], "weight": weight, "wsc": wsc} for c in range(B)]
    trace = bool(int(os.environ.get("BITLINEAR_TRACE", "0")))
    res = run_bass_kernel_spmd(
        nc, in_maps, list(range(B)), trace=trace, tmpdir=TRACE_DIR
    )
    _CACHE["last"] = res
    return np.stack([res.results[c]["y"] for c in range(B)], axis=0)


# revision 10
# speedup vs baseline: 2.2621x; 2.2621x over previous
"""BitLinear fake-quant GEMM on 8 trn2 NeuronCores, data-parallel over batch.

Per core: y[s,o] = round(clip(x/a_scale*127)) @ clip(round(w/w_scale),-1,1)^T
          * (w_scale * a_scale / 127),  a_scale = rowmax|x| + eps.

Quantized activations are integers |v|<=127 and weights are ternary, so a
bf16 matmul with fp32 PSUM accumulation is exact integer arithmetic.
"""

import os
import sys

import numpy as np

sys.path.insert(0, "/opt/trn_rl_repo")

import concourse.bacc as bacc
import concourse.mybir as mybir
import concourse.tile as tile
from concourse.bass_utils import run_bass_kernel_spmd

F32 = mybir.dt.float32
BF16 = mybir.dt.bfloat16
AF = mybir.ActivationFunctionType
ALU = mybir.AluOpType

B = 8      # batches == cores
S = 4096   # rows per core
D = 1024   # in features (contraction)
O = 1024   # out features
P = 128
GA = 4     # s-tiles per DMA group
KB = D // P
RND = 12582912.0  # 1.5*2**23: (z+RND)-RND == round-half-even(z) for |z|<2**22
EPS = 1e-8

_CACHE = {}
TRACE_DIR = None


def _build(s_rows=S):
    nt = s_rows // P
    ng = nt // GA
    nc = bacc.Bacc("TRN2", target_bir_lowering=False, debug=False)
    x_d = nc.dram_tensor("x", [s_rows, D], F32, kind="ExternalInput")
    w_d = nc.dram_tensor("weight", [O, D], F32, kind="ExternalInput")
    wsc_d = nc.dram_tensor("wsc", [1, 2], F32, kind="ExternalInput")
    y_d = nc.dram_tensor("y", [s_rows, O], F32, kind="ExternalOutput")
    xa, wa, sca, ya = x_d.ap(), w_d.ap(), wsc_d.ap(), y_d.ap()

    with tile.TileContext(nc) as tc:
        with (
            tc.tile_pool(name="wraw", bufs=1) as wraw_p,
            tc.tile_pool(name="wq", bufs=2) as wq_p,
            tc.tile_pool(name="wqT", bufs=1) as wqT_p,
            tc.tile_pool(name="xg", bufs=3) as xg_p,
            tc.tile_pool(name="stat", bufs=4) as stat_p,
            tc.tile_pool(name="quant", bufs=3) as q_p,
            tc.tile_pool(name="aqT", bufs=4) as aqT_p,
            tc.tile_pool(name="yout", bufs=3) as y_p,
            tc.tile_pool(name="psum", bufs=3, space="PSUM") as ps_p,
        ):
            # scalar broadcast: wsc = [1/w_scale, w_scale/127] -> all partitions
            wsc0 = wraw_p.tile([P, 2], F32, tag="wsc0")
            nc.sync.dma_start(out=wsc0[0:1, :], in_=sca[:, :])
            wscb = wraw_p.tile([P, 2], F32, tag="wscb")
            nc.gpsimd.partition_broadcast(wscb[:, :], wsc0[0:1, :], channels=P)
            recw_b = wscb[:, 0:1]
            ws127_b = wscb[:, 1:2]

            # weight: quantize to ternary bf16, then transpose to [i, o]
            wa3 = wa.rearrange("(a p) d -> p a d", p=P)
            wqT = wqT_p.tile([P, KB, O], BF16)  # [i-in-blk, i-blk, o]
            for k in range(KB):
                w_sb = wq_p.tile([P, D], F32, tag="wraw")
                nc.sync.dma_start(out=w_sb[:], in_=wa3[:, k, :])
                tw = wq_p.tile([P, D], F32, tag="tw")
                nc.scalar.activation(tw[:], w_sb[:], AF.Copy, bias=RND, scale=recw_b)
                tw2 = wq_p.tile([P, D], F32, tag="tw2")
                nc.vector.tensor_scalar(tw2[:], tw[:], RND, 1.0, ALU.subtract, ALU.min)
                wq = wq_p.tile([P, D], BF16, tag="wq")
                nc.vector.tensor_scalar(wq[:], tw2[:], -1.0, None, ALU.max)
                # batched xbar transpose: [128 o, 1024 i] -> i split over
                # (blk, part) in one instruction; exact i<->(blk,part) mapping
                # only needs to match the activation transpose below.
                nc.sync.dma_start_transpose(wqT[:, :, k * P:(k + 1) * P], wq[:])

            for g in range(ng):
                xg = xg_p.tile([P, GA, D], F32)
                nc.sync.dma_start(
                    out=xg[:],
                    in_=xa[g * GA * P:(g + 1) * GA * P, :].rearrange("(a p) d -> p a d", p=P),
                )
                st = stat_p.tile([P, GA], F32, tag="st")
                nc.vector.tensor_reduce(
                    st[:], xg[:], mybir.AxisListType.X, ALU.max, apply_absolute_value=True
                )
                ga_t = stat_p.tile([P, GA], F32, tag="ga")
                nc.vector.tensor_scalar(ga_t[:], st[:], EPS, None, ALU.add)
                rec = stat_p.tile([P, GA], F32, tag="rec")
                nc.vector.reciprocal(rec[:], ga_t[:])
                rec127 = stat_p.tile([P, GA], F32, tag="rec127")
                nc.vector.tensor_scalar(rec127[:], rec[:], 127.0, None, ALU.mult)
                epi = stat_p.tile([P, GA], F32, tag="epi")
                nc.vector.tensor_scalar(epi[:], ga_t[:], ws127_b, None, ALU.mult)

                for a in range(GA):
                    t = g * GA + a
                    tq = q_p.tile([P, D], F32, tag="tq")
                    nc.scalar.activation(
                        tq[:], xg[:, a, :], AF.Copy, bias=RND, scale=rec127[:, a:a + 1]
                    )
                    aq = q_p.tile([P, D], BF16, tag="aq")
                    nc.vector.tensor_scalar(aq[:], tq[:], RND, None, ALU.subtract)
                    aqT = aqT_p.tile([P, KB, P], BF16)
                    # all transposes on one engine: concurrent xbar transposes
                    # from two HWDGE queues corrupt data (shared-xbar hazard)
                    nc.sync.dma_start_transpose(aqT[:], aq[:])
                    yt = ps_p.tile([P, O], F32)
                    for b2 in range(KB):
                        nc.tensor.matmul(
                            yt[:, 0:512], aqT[:, b2, :], wqT[:, b2, 0:512],
                            start=(b2 == 0), stop=(b2 == KB - 1),
                        )
                        nc.tensor.matmul(
                            yt[:, 512:1024], aqT[:, b2, :], wqT[:, b2, 512:1024],
                            start=(b2 == 0), stop=(b2 == KB - 1),
                        )
                    ysb = y_p.tile([P, O], F32)
                    nc.scalar.activation(ysb[:], yt[:], AF.Copy, bias=0.0, scale=epi[:, a:a + 1])
                    nc.sync.dma_start(out=ya[t * P:(t + 1) * P, :], in_=ysb[:])
    nc.compile()
    return nc


def _scales(weight):
    # w_scale in fp64 then rounded, mirroring fp32 `mean(|w|) + eps` as closely
    # as any fp32 summation order allows.
    m = np.abs(weight.astype(np.float64)).mean()
    ws = np.float32(np.float32(m) + np.float32(EPS))
    recw = np.float32(1.0 / np.float64(ws))
    ws127 = np.float32(np.float64(ws) / 127.0)
    return np.array([[recw, ws127]], dtype=np.float32)


def kernel(x, weight):
    x = np.ascontiguousarray(np.asarray(x), dtype=np.float32)
    weight = np.ascontiguousarray(np.asarray(weight), dtype=np.float32)
    assert x.shape == (B, S, D) and weight.shape == (O, D)
    nc = _CACHE.get("nc")
    if nc is None:
        nc = _CACHE["nc"] = _build()
    wsc = _scales(weight)
    in_maps = [{"x": x[c], "weight": weight, "wsc": wsc} for c in range(B)]
    trace = bool(int(os.environ.get("BITLINEAR_TRACE", "0")))
    res = run_bass_kernel_spmd(
        nc, in_maps, list(range(B)), trace=trace, tmpdir=TRACE_DIR
    )
    _CACHE["last"] = res
    return np.stack([res.results[c]["y"] for c in range(B)], axis=0)
